# revision 21
# baseline (speedup 1.0000x reference)
"""Multi-head attention (B=16, N=1024, E=768, H=12, D=64) on 8 TRN2 NeuronCores.

Strategy: data-parallel over batch (2 batches per core, no collectives).

v2 design (vs baseline): head-PAIR attention. Heads (2j, 2j+1) live on
partition halves 0:64 / 64:128 of feature tile j, so their D=64 score
matmuls target disjoint PE row-groups and run concurrently (measured
1.96x over same-half sequencing). Other changes:
  - x transposes in bf16 (cast first), 6 per token tile into ONE bitcast
    PSUM bank, drained by a single strided DVE copy.
  - scores psum: one [128,1024] f32 tile per head of the pair (4 banks),
    exp'd per head per key-tile; Pt per head in SBUF (pool of 3).
  - PV unchanged math (ones-column denominator trick) but pieces for the
    whole pair interleave into the next pair's score slots.
  - reciprocal computed on a [16,128] layout (DRAM-side rearrange) - was
    [6,1024], 7.8us per call; now ~1us for the same work.
  - v-bias adds batched into one strided DVE op per (tt, oc).
  - schedule: batch b attention hosts prep(b+1) fillers; vproj(b+1) runs
    under the pre-scoring of pair0(b+1) (the "bridge"); oproj(b-1) fills
    attention(b); normalize per pair, two pairs behind.
"""

from contextlib import ExitStack

import numpy as np

import concourse.bass as bass
import concourse.mybir as mybir
import concourse.tile as tile
from concourse import bacc
from concourse.bass_utils import run_bass_kernel_spmd
from concourse.masks import make_identity

F32 = mybir.dt.float32
BF16 = mybir.dt.bfloat16
AF = mybir.ActivationFunctionType
OP = mybir.AluOpType

P = 128
E = 768          # embed dim
H = 12           # heads
D = 64           # head dim
KO = E // P      # 6 contraction subtiles over embed
NP = H // 2      # head pairs
B_FULL = 16
N_FULL = 1024
N_CORES = 8
BPC = B_FULL // N_CORES  # batches per core


def _body(ctx, tc, out_d, x_d, wqkv_d, bqkv_d, wproj_d, bproj_d, n_batch, N,
          dbg=None):
    nc = tc.nc
    TT = N // P                 # key tiles per batch
    CS = min(512, N)            # query-chunk size
    NCH = N // CS               # query chunks

    const = ctx.enter_context(tc.tile_pool(name="const", bufs=1))
    xt_pool = ctx.enter_context(tc.tile_pool(name="xt", bufs=1))
    qk_pool = ctx.enter_context(tc.tile_pool(name="qk", bufs=2))
    v_pool = ctx.enter_context(tc.tile_pool(name="v", bufs=1))
    pt_pool = ctx.enter_context(tc.tile_pool(name="pt", bufs=3))
    ot_pool = ctx.enter_context(tc.tile_pool(name="ot", bufs=2))
    small = ctx.enter_context(tc.tile_pool(name="small", bufs=2))
    stage = ctx.enter_context(tc.tile_pool(name="stage", bufs=2))
    nrm = ctx.enter_context(tc.tile_pool(name="nrm", bufs=1))
    nrm2 = ctx.enter_context(tc.tile_pool(name="nrm2", bufs=2))
    dram = ctx.enter_context(tc.tile_pool(name="dram", bufs=2, space="DRAM"))
    osb_pool = ctx.enter_context(tc.tile_pool(name="osb", bufs=2))
    psS = ctx.enter_context(tc.tile_pool(name="psS", bufs=1, space="PSUM"))
    psB = ctx.enter_context(tc.tile_pool(name="psB", bufs=2, space="PSUM"))
    psV = ctx.enter_context(tc.tile_pool(name="psV", bufs=2, space="PSUM"))

    # ---------------- constants ----------------
    identb = const.tile([P, P], BF16, tag="identb")
    make_identity(nc, identb)
    # warm the exp table while startup DMAs stream
    warm = const.tile([1, 1], F32, tag="warm")
    nc.scalar.activation(warm[:], identb[0:1, 0:1], AF.Exp, scale=1.0)

    wqkv_sb = const.tile([P, KO, 3 * E], BF16, tag="wqkv")
    wproj_sb = const.tile([P, KO, E], BF16, tag="wproj")
    bqk_sb = const.tile([P, 2 * KO], F32, tag="bqk")
    bv_sb = const.tile([P, E], BF16, tag="bv")
    bp_sb = const.tile([P, E], BF16, tag="bp")
    wq_r = wqkv_d.rearrange("(ko p) n -> p ko n", p=P)
    wp_r = wproj_d.rearrange("(ko p) n -> p ko n", p=P)

    def bias_chunk():
        # q/k bias as per-partition columns: feature f = m*128+p -> [128,12]
        nc.sync.dma_start(
            bqk_sb[:], bqkv_d[: 2 * E].rearrange("(m p) -> p m", p=P)
        )
        # v / proj bias replicated across partitions (free-dim varying)
        nc.gpsimd.dma_start(
            bv_sb[:], bqkv_d[2 * E : 3 * E].partition_broadcast(P)
        )
        nc.gpsimd.dma_start(bp_sb[:], bproj_d.partition_broadcast(P))

    def wload(m, eng=None):
        """Load one 128-column slice of w_qkv (m<18) or w_proj (m>=18),
        all ko at once, so the consumer matmul chain unblocks after ONE
        small DMA. `eng`: which DMA queue (sync/gpsimd/scalar)."""
        def emit():
            e = eng if eng is not None else nc.sync
            t = stage.tile([P, KO, P], F32, tag="ws", name="ws")
            if m < 18:
                e.dma_start(t[:], wq_r[:, :, m * P : (m + 1) * P])
                nc.vector.tensor_copy(
                    wqkv_sb[:, :, m * P : (m + 1) * P], t[:]
                )
            else:
                mo = m - 18
                e.dma_start(t[:], wp_r[:, :, mo * P : (mo + 1) * P])
                nc.vector.tensor_copy(
                    wproj_sb[:, :, mo * P : (mo + 1) * P], t[:]
                )

        return emit

    # Per-batch state handles
    xT = [None] * n_batch
    qkT = [None] * n_batch
    v_sb = [None] * n_batch
    OT = [None] * n_batch
    sums_of = [None] * n_batch
    rdram_of = [None] * n_batch

    # ---------------- phase emitters (closures) -----------------------
    def xprep_chunk(b, tt):
        def emit():
            if tt == 0:
                xT[b] = xt_pool.tile([P, KO, N], BF16, tag="xT", name="xT")
            xtmp = stage.tile([P, E], F32, tag="ws", name="ws")
            nc.sync.dma_start(xtmp[:], x_d[b, tt * P : (tt + 1) * P, :])
            xb = stage.tile([P, E], BF16, tag="xb", name="xb")
            nc.vector.tensor_copy(xb[:], xtmp[:])
            ps = psB.tile([P, CS], F32, tag="pb", name="pb")
            psv = ps.bitcast(BF16)
            for ko in range(KO):
                nc.tensor.transpose(
                    psv[:, ko * P : (ko + 1) * P],
                    xb[:, ko * P : (ko + 1) * P],
                    identb,
                )
            nc.vector.tensor_copy(
                xT[b][:, :, tt * P : (tt + 1) * P],
                psv[:, : KO * P].rearrange("p (ko t) -> p ko t", t=P),
            )

        return emit

    def qk_chunk(b, m, ch):
        def emit():
            if m == 0 and ch == 0:
                qkT[b] = qk_pool.tile([P, 2 * KO, N], BF16, tag="qkT", name="qkT")
            ps = psB.tile([P, CS], F32, tag="pb", name="pb")
            for ko in range(KO):
                nc.tensor.matmul(
                    ps[:, :CS],
                    wqkv_sb[:, ko, m * P : (m + 1) * P],
                    xT[b][:, ko, ch * CS : (ch + 1) * CS],
                    start=(ko == 0),
                    stop=(ko == KO - 1),
                )
            nc.vector.tensor_tensor(
                qkT[b][:, m, ch * CS : (ch + 1) * CS],
                ps[:, :CS],
                bqk_sb[:, m : m + 1].to_broadcast([P, CS]),
                OP.add,
            )

        return emit

    def v_chunk(b, tt, oc):
        def emit():
            if tt == 0 and oc == 0:
                v_sb[b] = v_pool.tile([P, TT, H * 65], BF16, tag="v", name="v")
                ones_cols = v_sb[b].rearrange("p t (h c) -> p t h c", c=65)[
                    :, :, :, 64
                ]
                nc.vector.memset(ones_cols, 1.0)
            ocs = 512 if oc == 0 else 256
            nh = 8 if oc == 0 else 4
            ps = psB.tile([P, CS], F32, tag="pb", name="pb")
            for ko in range(KO):
                nc.tensor.matmul(
                    ps[:, :ocs],
                    xT[b][:, ko, tt * P : (tt + 1) * P],
                    wqkv_sb[:, ko, 2 * E + oc * 512 : 2 * E + oc * 512 + ocs],
                    start=(ko == 0),
                    stop=(ko == KO - 1),
                )
            # batched bias add: one strided op for all nh heads
            nc.vector.tensor_tensor(
                v_sb[b][:, tt, :].rearrange("p (h c) -> p h c", c=65)[
                    :, oc * 8 : oc * 8 + nh, :64
                ],
                ps[:, :ocs].rearrange("p (h c) -> p h c", c=64),
                bv_sb[:, oc * 512 : oc * 512 + ocs].rearrange(
                    "p (h c) -> p h c", c=64
                ),
                OP.add,
            )

        return emit

    _osb_state = {}

    def oproj_chunk(b, tt, oc):
        def emit():
            if oc == 0:
                _osb_state[(b, tt)] = osb_pool.tile(
                    [P, E], F32, tag="osb", name="osb"
                )
            osb = _osb_state[(b, tt)]
            ocs = 512 if oc == 0 else 256
            ps = psB.tile([P, CS], F32, tag="pb", name="pb")
            for ko in range(KO):
                nc.tensor.matmul(
                    ps[:, :ocs],
                    OT[b][:, ko, tt * P : (tt + 1) * P],
                    wproj_sb[:, ko, oc * 512 : oc * 512 + ocs],
                    start=(ko == 0),
                    stop=(ko == KO - 1),
                )
            nc.vector.tensor_tensor(
                osb[:, oc * 512 : oc * 512 + ocs],
                ps[:, :ocs],
                bp_sb[:, oc * 512 : oc * 512 + ocs],
                OP.add,
            )
            if oc == 1:
                nc.sync.dma_start(
                    out_d[b, tt * P : (tt + 1) * P, :], osb[:]
                )

        return emit

    # ---------------- attention (head pairs) ---------------------------
    def pair_scores_kt(b, pair, kt, psA, psB2, Pt0, Pt1):
        """4 score matmuls for head pair `pair` at key tile kt, alternating
        PE row-halves, then the two exps."""
        ft = pair
        for ch in range(NCH):
            for hh, ps in ((0, psA), (1, psB2)):
                pr = hh * 64
                nc.tensor.matmul(
                    ps[:, ch * CS : (ch + 1) * CS],
                    qkT[b][pr : pr + 64, KO + ft, kt * P : (kt + 1) * P],
                    qkT[b][pr : pr + 64, ft, ch * CS : (ch + 1) * CS],
                    start=True,
                    stop=True,
                )
        nc.scalar.activation(Pt0[:, kt, :], psA[:, :N], AF.Exp, scale=0.125)
        nc.scalar.activation(Pt1[:, kt, :], psB2[:, :N], AF.Exp, scale=0.125)

    def make_pv_pieces(b, pair, Pt0, Pt1):
        """PV matmuls + epilogues for both heads of `pair`, as closures
        consumed during the NEXT pair's score slots."""
        pieces = []
        state = {}

        def mk_mm(h, ch, g, Pt):
            def emit():
                if g == 0:
                    state[(h, ch)] = psV.tile(
                        [P, CS], F32, tag="po", name="po"
                    )
                po = state[(h, ch)]
                for kt in (2 * g, 2 * g + 1):
                    nc.tensor.matmul(
                        po[:65, :CS],
                        v_sb[b][:, kt, h * 65 : (h + 1) * 65],
                        Pt[:, kt, ch * CS : (ch + 1) * CS],
                        start=(kt == 0),
                        stop=(kt == TT - 1),
                    )

            return emit

        def mk_tail(h, ch, ft, pr):
            def emit():
                po = state[(h, ch)]
                otmp = small.tile([65, CS], BF16, tag="otmp", name="otmp")
                nc.vector.tensor_copy(otmp[:, :CS], po[0:65, :CS])
                nc.sync.dma_start(
                    sums_of[b][8 * h + 4 * ch : 8 * h + 4 * ch + 4, :],
                    otmp[64:65, :CS],
                )
                nc.sync.dma_start(
                    OT[b][pr : pr + 64, ft, ch * CS : (ch + 1) * CS],
                    otmp[0:64, :CS],
                )

            return emit

        for hh, Pt in ((0, Pt0), (1, Pt1)):
            h = 2 * pair + hh
            ft, pr = pair, hh * 64
            # ch interleaved inside each kt-group so Pt frees in kt order
            # (the next pair's exp WAR-waits on these reads)
            for g in range(TT // 2):
                for ch in range(NCH):
                    pieces.append(mk_mm(h, ch, g, Pt))
            for ch in range(NCH):
                pieces.append(mk_tail(h, ch, ft, pr))
        return pieces

    def normalize_head(b, h):
        """Closures: reciprocal of head h's denominators ([8,128] layout),
        then the OT multiply."""

        def recip():
            sums_sb = nrm.tile([8, P], BF16, tag="sums_sb", name="sums_sb")
            nc.sync.dma_start(sums_sb[:], sums_of[b][8 * h : 8 * h + 8, :])
            rsum = nrm.tile([8, P], BF16, tag="rsum", name="rsum")
            with nc.allow_low_precision(reason="softmax denom recip in bf16"):
                nc.vector.reciprocal(rsum[:], sums_sb[:])
            nc.sync.dma_start(rdram_of[b][8 * h : 8 * h + 8, :], rsum[:])

        def mult():
            ft, pr = h // 2, (h % 2) * 64
            rb = nrm2.tile([P, N], BF16, tag="rb", name="rb")
            nc.gpsimd.dma_start(
                rb[pr : pr + 64, :],
                rdram_of[b][8 * h : 8 * h + 8, :].partition_broadcast(64),
            )
            dst = OT[b][pr : pr + 64, ft, :]
            nc.vector.tensor_tensor(dst, dst, rb[pr : pr + 64, :], OP.mult)

        return [recip, mult]

    def alloc_attn_state(b):
        OT[b] = ot_pool.tile([P, KO, N], BF16, tag="OT", name="OT")
        sums_of[b] = dram.tile([8 * H, P], BF16, tag="sums", name="sums")
        rdram_of[b] = dram.tile([8 * H, P], BF16, tag="rdram", name="rdram")

    def pre_score_pair0(b, fill_iter):
        """Score pair 0 of batch b; pull fillers between kt slots."""
        psA = psS.tile([P, N], F32, tag="sA", name="sA")
        psB2 = psS.tile([P, N], F32, tag="sB", name="sB")
        Pt0 = pt_pool.tile([P, TT, N], BF16, tag="Pt", name="Pt")
        Pt1 = pt_pool.tile([P, TT, N], BF16, tag="Pt", name="Pt")
        for kt in range(TT):
            pair_scores_kt(b, 0, kt, psA, psB2, Pt0, Pt1)
            for _ in range(4):
                nxt = next(fill_iter, None)
                if nxt is not None:
                    nxt()
        return (psA, psB2, Pt0, Pt1)

    def emit_attention(b, hot, fillers, forced_map):
        """Pairs 1..NP-1 of batch b. `hot` = psum tiles + Pt of pair 0.
        fillers: list of closures, consumed to keep PE busy.
        forced_map: pair -> list of closures that MUST be consumed during
        that pair's slots (normalize work)."""
        psA, psB2, Pt0, Pt1 = hot
        fi = [0]

        def pop_filler():
            if fi[0] < len(fillers):
                fillers[fi[0]]()
                fi[0] += 1

        pieces = make_pv_pieces(b, 0, Pt0, Pt1)
        for pair in range(1, NP):
            Pt0 = pt_pool.tile([P, TT, N], BF16, tag="Pt", name="Pt")
            Pt1 = pt_pool.tile([P, TT, N], BF16, tag="Pt", name="Pt")
            npv = len(pieces)
            forced = list(forced_map.get(pair, ()))
            nfo = len(forced)
            pi = 0
            # spread remaining fillers evenly over remaining pairs
            nf_target = (len(fillers) - fi[0]) // (NP - pair)
            for kt in range(TT):
                pair_scores_kt(b, pair, kt, psA, psB2, Pt0, Pt1)
                quota = ((kt + 1) * npv) // TT - (kt * npv) // TT
                for _ in range(quota):
                    pieces[pi]()
                    pi += 1
                fq = ((kt + 1) * nfo) // TT - (kt * nfo) // TT
                for _ in range(fq):
                    forced.pop(0)()
                pq = ((kt + 1) * nf_target) // TT - (kt * nf_target) // TT
                for _ in range(pq):
                    pop_filler()
            while pi < npv:
                pieces[pi]()
                pi += 1
            pieces = make_pv_pieces(b, pair, Pt0, Pt1)
        # drain: PV of the last pair, normalize interleaved as soon as the
        # per-head tails land, remaining fillers last
        fNP = list(forced_map.get(NP, ()))
        fLast = list(forced_map.get(NP + 1, ()))
        for i, p in enumerate(pieces):
            p()
            if fNP:
                fNP.pop(0)()
            if i == 9 and len(fLast) >= 2:
                fLast.pop(0)()
                fLast.pop(0)()
            if i % 2 == 1:
                pop_filler()
        for c in fLast:
            c()
        while fi[0] < len(fillers):
            pop_filler()

    # ---------------- top-level schedule ------------------------------
    # batch 0 x prep first (x DMAs own the sync queue, PE transposes
    # start as soon as the first load+cast lands)
    bias_chunk()
    for tt in range(TT):
        xprep_chunk(0, tt)()
    # weights + qk projections for pairs 0 and 1: each m-tile unblocks
    # after one small [128,6,128] DMA on the (startup-idle) scalar queue
    for m in (0, KO, 1, KO + 1):
        wload(m, eng=nc.scalar)()
        for ch in range(NCH):
            qk_chunk(0, m, ch)()

    # order matters: all of v(0) must be EMITTED during pair0 pre-scoring
    # (32 slots), before PV(pair0) pieces read it in attention pair 1
    startup_fill = (
        [wload(2), qk_chunk(0, 2, 0), qk_chunk(0, 2, 1),
         wload(KO + 2), qk_chunk(0, KO + 2, 0), qk_chunk(0, KO + 2, 1)]
        + [wload(m) for m in range(12, 18)]
        + [v_chunk(0, tt, oc) for tt in range(TT) for oc in range(2)]
        + [wload(3), qk_chunk(0, 3, 0), qk_chunk(0, 3, 1),
           wload(KO + 3), qk_chunk(0, KO + 3, 0), qk_chunk(0, KO + 3, 1)]
        + [c for ft in (4, 5) for c in (
            wload(ft), qk_chunk(0, ft, 0), qk_chunk(0, ft, 1),
            wload(KO + ft), qk_chunk(0, KO + ft, 0), qk_chunk(0, KO + ft, 1))]
        + [wload(m) for m in range(18, 24)]
    )
    sf_iter = iter(startup_fill)
    alloc_attn_state(0)
    hot0 = pre_score_pair0(0, sf_iter)

    for b in range(n_batch):
        fillers = list(sf_iter) if b == 0 else []
        sf_iter = iter(())
        if b + 1 < n_batch:
            fillers += [xprep_chunk(b + 1, tt) for tt in range(TT)]
            fillers += [
                qk_chunk(b + 1, m, ch)
                for ft in range(KO)
                for m in (ft, KO + ft)
                for ch in range(NCH)
            ]
        if b > 0:
            fillers += [
                oproj_chunk(b - 1, tt, oc) for tt in range(TT) for oc in range(2)
            ]
        # normalize pair p's heads during pair p+2's slots
        forced_map = {}
        for pair in range(NP):
            forced_map.setdefault(min(pair + 2, NP + 1), []).extend(
                normalize_head(b, 2 * pair) + normalize_head(b, 2 * pair + 1)
            )
        emit_attention(b, hot0, fillers, forced_map)
        if b + 1 < n_batch:
            # bridge: pre-score pair0 of b+1 while vproj(b+1) streams
            alloc_attn_state(b + 1)
            bridge = [v_chunk(b + 1, tt, oc) for tt in range(TT) for oc in range(2)]
            hot0 = pre_score_pair0(b + 1, iter(bridge))
    # tail: out projection of the last batch, split so the ko=0..4 streams
    # run inside the last heads' normalize latency window and only the
    # ko=5 accumulate (needs heads 10/11 normalized) waits on it
    bl = n_batch - 1
    tail_ps = {}

    def op_part1(tt, oc):
        ocs = 512 if oc == 0 else 256
        ps = psB.tile([P, CS], F32, tag="pb", name="pb")
        tail_ps[(tt, oc)] = ps
        for ko in range(KO - 1):
            nc.tensor.matmul(
                ps[:, :ocs],
                OT[bl][:, ko, tt * P : (tt + 1) * P],
                wproj_sb[:, ko, oc * 512 : oc * 512 + ocs],
                start=(ko == 0),
                stop=False,
            )

    def op_part2(tt, oc):
        if oc == 0:
            _osb_state[(bl, tt)] = osb_pool.tile([P, E], F32, tag="osb", name="osb")
        osb = _osb_state[(bl, tt)]
        ocs = 512 if oc == 0 else 256
        ps = tail_ps[(tt, oc)]
        nc.tensor.matmul(
            ps[:, :ocs],
            OT[bl][:, KO - 1, tt * P : (tt + 1) * P],
            wproj_sb[:, KO - 1, oc * 512 : oc * 512 + ocs],
            start=False,
            stop=True,
        )
        nc.vector.tensor_tensor(
            osb[:, oc * 512 : oc * 512 + ocs],
            ps[:, :ocs],
            bp_sb[:, oc * 512 : oc * 512 + ocs],
            OP.add,
        )
        if oc == 1:
            nc.sync.dma_start(out_d[bl, tt * P : (tt + 1) * P, :], osb[:])

    op_part1(0, 0)
    op_part1(0, 1)
    for tt in range(TT):
        op_part2(tt, 0)
        op_part2(tt, 1)
        if tt + 1 < TT:
            op_part1(tt + 1, 0)
            op_part1(tt + 1, 1)
    if dbg is not None:
        nc.sync.dma_start(dbg["qkT"], qkT[0][:])
        nc.sync.dma_start(dbg["v"], v_sb[0][:])
        nc.sync.dma_start(dbg["ot"], OT[0][:])
        nc.sync.dma_start(dbg["sums"], sums_of[0][:])


def build_graph(n_batch=BPC, N=N_FULL, n_cores=N_CORES, debug=False):
    nc = bacc.Bacc(
        "TRN2", target_bir_lowering=False, debug=False, num_devices=n_cores
    )
    x_d = nc.dram_tensor("x", [n_batch, N, E], F32, kind="ExternalInput").ap()
    wqkv_d = nc.dram_tensor("w_qkv", [E, 3 * E], F32, kind="ExternalInput").ap()
    bqkv_d = nc.dram_tensor("b_qkv", [3 * E], F32, kind="ExternalInput").ap()
    wproj_d = nc.dram_tensor("w_proj", [E, E], F32, kind="ExternalInput").ap()
    bproj_d = nc.dram_tensor("b_proj", [E], F32, kind="ExternalInput").ap()
    out_d = nc.dram_tensor("out", [n_batch, N, E], F32, kind="ExternalOutput").ap()

    dbg = None
    if debug:
        dbg = {
            "qkT": nc.dram_tensor("dbg_qkT", [P, 2 * KO, N], BF16, kind="ExternalOutput").ap(),
            "v": nc.dram_tensor("dbg_v", [P, N_FULL // P, H * 65], BF16, kind="ExternalOutput").ap(),
            "ot": nc.dram_tensor("dbg_ot", [P, KO, N], BF16, kind="ExternalOutput").ap(),
            "sums": nc.dram_tensor("dbg_sums", [8 * H, P], BF16, kind="ExternalOutput").ap(),
        }
    with tile.TileContext(nc) as tc, ExitStack() as ctx:
        _body(ctx, tc, out_d, x_d, wqkv_d, bqkv_d, wproj_d, bproj_d, n_batch, N, dbg=dbg)
    nc.compile()
    return nc


_NC_CACHE = {}


def _get_graph():
    if "nc" not in _NC_CACHE:
        _NC_CACHE["nc"] = build_graph()
    return _NC_CACHE["nc"]


def run_on_hw(x, w_qkv, b_qkv, w_proj, b_proj, trace=False):
    nc = _get_graph()
    x = np.ascontiguousarray(np.asarray(x, dtype=np.float32))
    shared = {
        "w_qkv": np.ascontiguousarray(np.asarray(w_qkv, dtype=np.float32)),
        "b_qkv": np.ascontiguousarray(np.asarray(b_qkv, dtype=np.float32)),
        "w_proj": np.ascontiguousarray(np.asarray(w_proj, dtype=np.float32)),
        "b_proj": np.ascontiguousarray(np.asarray(b_proj, dtype=np.float32)),
    }
    in_maps = [
        {"x": x[i * BPC : (i + 1) * BPC], **shared} for i in range(N_CORES)
    ]
    res = run_bass_kernel_spmd(
        nc, in_maps, core_ids=list(range(N_CORES)), trace=trace
    )
    out = np.concatenate([r["out"] for r in res.results], axis=0)
    return out, res


def kernel(x, w_qkv, b_qkv, w_proj, b_proj):
    out, _ = run_on_hw(x, w_qkv, b_qkv, w_proj, b_proj)
    return out


# revision 25
# speedup vs baseline: 1.1999x; 1.1999x over previous
"""Multi-head attention (B=16, N=1024, E=768, H=12, D=64) on 8 TRN2 NeuronCores.

Strategy: data-parallel over batch (2 batches per core, no collectives).
Per-core kernel: qkv = x @ w_qkv + b, per-head attention, out projection.
Measured ~396 us/core (from 557 us for the previous interleaved kernel).

Design:
  - head-PAIR attention: heads (2j, 2j+1) live on partition halves
    0:64 / 64:128 of feature tile j, so their D=64 score matmuls hit
    disjoint PE row-groups and pipeline/run concurrently (measured 1.96x
    over same-half sequencing: LDWEIGHTS+drain of one half hide under the
    other half's stream).
  - x^T built by PE transposes in bf16 (DVE cast first); all 6 ko blocks
    of a token tile go into ONE PSUM bank via a bf16 bitcast view and are
    drained by a single strided DVE copy.
  - scores: one [128,1024] f32 PSUM tile per head of the pair (4 banks
    total); exp fused into the PSUM->SBUF Pt copy on the scalar engine
    (scale=0.125 via the free ACT affine); softmax denominator comes free
    from an all-ones 65th column in each v tile (row 64 of the PV psum).
  - PV pieces for the whole pair are consumed inside the next pair's
    ACT-paced score slots; ch is interleaved inside each kt group so the
    Pt buffers free in kt order for the next pair's exp WAR.
  - normalization: reciprocal on an [8,128] all-partition layout (was
    [6,1024): 7.8us -> ~1us), DRAM round-trip with plain-slice APs only
    (rearranged DRAM views broke dependency tracking), gpsimd partition
    broadcast, one DVE multiply per head.
  - weights stream in per-m [128, 6ko, 128] slices so each qk projection
    chain unblocks after one small DMA; the four startup-critical slices
    ride the idle scalar HWDGE queue.
  - schedule: batch b attention hosts prep(b+1) fillers; vproj(b+1) runs
    under the pre-scoring of pair0(b+1) (the bridge); oproj(b-1) fills
    attention(b); per-head normalize lands two pairs behind its PV; the
    tail out-projection is split ko0-4 / ko5 so only the final accumulate
    waits on the last heads' normalize.
"""

from contextlib import ExitStack

import numpy as np

import concourse.bass as bass
import concourse.mybir as mybir
import concourse.tile as tile
from concourse import bacc
from concourse.bass_utils import run_bass_kernel_spmd
from concourse.masks import make_identity

F32 = mybir.dt.float32
BF16 = mybir.dt.bfloat16
AF = mybir.ActivationFunctionType
OP = mybir.AluOpType

P = 128
E = 768          # embed dim
H = 12           # heads
D = 64           # head dim
KO = E // P      # 6 contraction subtiles over embed
NP = H // 2      # head pairs
B_FULL = 16
N_FULL = 1024
N_CORES = 8
BPC = B_FULL // N_CORES  # batches per core


def _body(ctx, tc, out_d, x_d, wqkv_d, bqkv_d, wproj_d, bproj_d, n_batch, N,
          dbg=None):
    nc = tc.nc
    TT = N // P                 # key tiles per batch
    CS = min(512, N)            # query-chunk size
    NCH = N // CS               # query chunks

    const = ctx.enter_context(tc.tile_pool(name="const", bufs=1))
    xt_pool = ctx.enter_context(tc.tile_pool(name="xt", bufs=1))
    qk_pool = ctx.enter_context(tc.tile_pool(name="qk", bufs=2))
    v_pool = ctx.enter_context(tc.tile_pool(name="v", bufs=1))
    pt_pool = ctx.enter_context(tc.tile_pool(name="pt", bufs=3))
    ot_pool = ctx.enter_context(tc.tile_pool(name="ot", bufs=2))
    small = ctx.enter_context(tc.tile_pool(name="small", bufs=2))
    stage = ctx.enter_context(tc.tile_pool(name="stage", bufs=2))
    nrm = ctx.enter_context(tc.tile_pool(name="nrm", bufs=1))
    nrm2 = ctx.enter_context(tc.tile_pool(name="nrm2", bufs=2))
    dram = ctx.enter_context(tc.tile_pool(name="dram", bufs=2, space="DRAM"))
    osb_pool = ctx.enter_context(tc.tile_pool(name="osb", bufs=2))
    psS = ctx.enter_context(tc.tile_pool(name="psS", bufs=1, space="PSUM"))
    psB = ctx.enter_context(tc.tile_pool(name="psB", bufs=2, space="PSUM"))
    psV = ctx.enter_context(tc.tile_pool(name="psV", bufs=2, space="PSUM"))

    # ---------------- constants ----------------
    identb = const.tile([P, P], BF16, tag="identb")
    make_identity(nc, identb)
    # warm the exp table while startup DMAs stream
    warm = const.tile([1, 1], F32, tag="warm")
    nc.scalar.activation(warm[:], identb[0:1, 0:1], AF.Exp, scale=1.0)

    wqkv_sb = const.tile([P, KO, 3 * E], BF16, tag="wqkv")
    wproj_sb = const.tile([P, KO, E], BF16, tag="wproj")
    bqk_sb = const.tile([P, 2 * KO], F32, tag="bqk")
    bv_sb = const.tile([P, E], BF16, tag="bv")
    bp_sb = const.tile([P, E], BF16, tag="bp")
    wq_r = wqkv_d.rearrange("(ko p) n -> p ko n", p=P)
    wp_r = wproj_d.rearrange("(ko p) n -> p ko n", p=P)

    def bias_chunk():
        # q/k bias as per-partition columns: feature f = m*128+p -> [128,12]
        nc.sync.dma_start(
            bqk_sb[:], bqkv_d[: 2 * E].rearrange("(m p) -> p m", p=P)
        )
        # v / proj bias replicated across partitions (free-dim varying)
        nc.gpsimd.dma_start(
            bv_sb[:], bqkv_d[2 * E : 3 * E].partition_broadcast(P)
        )
        nc.gpsimd.dma_start(bp_sb[:], bproj_d.partition_broadcast(P))

    def wload(m, eng=None):
        """Load one 128-column slice of w_qkv (m<18) or w_proj (m>=18),
        all ko at once, so the consumer matmul chain unblocks after ONE
        small DMA. `eng`: which DMA queue (sync/gpsimd/scalar)."""
        def emit():
            e = eng if eng is not None else nc.sync
            t = stage.tile([P, KO, P], F32, tag="ws", name="ws")
            if m < 18:
                e.dma_start(t[:], wq_r[:, :, m * P : (m + 1) * P])
                nc.vector.tensor_copy(
                    wqkv_sb[:, :, m * P : (m + 1) * P], t[:]
                )
            else:
                mo = m - 18
                e.dma_start(t[:], wp_r[:, :, mo * P : (mo + 1) * P])
                nc.vector.tensor_copy(
                    wproj_sb[:, :, mo * P : (mo + 1) * P], t[:]
                )

        return emit

    # Per-batch state handles
    xT = [None] * n_batch
    qkT = [None] * n_batch
    v_sb = [None] * n_batch
    OT = [None] * n_batch
    sums_of = [None] * n_batch
    rdram_of = [None] * n_batch

    # ---------------- phase emitters (closures) -----------------------
    def xprep_chunk(b, tt):
        def emit():
            if tt == 0:
                xT[b] = xt_pool.tile([P, KO, N], BF16, tag="xT", name="xT")
            xtmp = stage.tile([P, E], F32, tag="ws", name="ws")
            nc.sync.dma_start(xtmp[:], x_d[b, tt * P : (tt + 1) * P, :])
            xb = stage.tile([P, E], BF16, tag="xb", name="xb")
            nc.vector.tensor_copy(xb[:], xtmp[:])
            ps = psB.tile([P, CS], F32, tag="pb", name="pb")
            psv = ps.bitcast(BF16)
            for ko in range(KO):
                nc.tensor.transpose(
                    psv[:, ko * P : (ko + 1) * P],
                    xb[:, ko * P : (ko + 1) * P],
                    identb,
                )
            nc.vector.tensor_copy(
                xT[b][:, :, tt * P : (tt + 1) * P],
                psv[:, : KO * P].rearrange("p (ko t) -> p ko t", t=P),
            )

        return emit

    def qk_chunk(b, m, ch):
        def emit():
            if m == 0 and ch == 0:
                qkT[b] = qk_pool.tile([P, 2 * KO, N], BF16, tag="qkT", name="qkT")
            ps = psB.tile([P, CS], F32, tag="pb", name="pb")
            for ko in range(KO):
                nc.tensor.matmul(
                    ps[:, :CS],
                    wqkv_sb[:, ko, m * P : (m + 1) * P],
                    xT[b][:, ko, ch * CS : (ch + 1) * CS],
                    start=(ko == 0),
                    stop=(ko == KO - 1),
                )
            nc.vector.tensor_tensor(
                qkT[b][:, m, ch * CS : (ch + 1) * CS],
                ps[:, :CS],
                bqk_sb[:, m : m + 1].to_broadcast([P, CS]),
                OP.add,
            )

        return emit

    def v_chunk(b, tt, oc):
        def emit():
            if tt == 0 and oc == 0:
                v_sb[b] = v_pool.tile([P, TT, H * 65], BF16, tag="v", name="v")
                ones_cols = v_sb[b].rearrange("p t (h c) -> p t h c", c=65)[
                    :, :, :, 64
                ]
                nc.vector.memset(ones_cols, 1.0)
            ocs = 512 if oc == 0 else 256
            nh = 8 if oc == 0 else 4
            ps = psB.tile([P, CS], F32, tag="pb", name="pb")
            for ko in range(KO):
                nc.tensor.matmul(
                    ps[:, :ocs],
                    xT[b][:, ko, tt * P : (tt + 1) * P],
                    wqkv_sb[:, ko, 2 * E + oc * 512 : 2 * E + oc * 512 + ocs],
                    start=(ko == 0),
                    stop=(ko == KO - 1),
                )
            # batched bias add: one strided op for all nh heads
            nc.vector.tensor_tensor(
                v_sb[b][:, tt, :].rearrange("p (h c) -> p h c", c=65)[
                    :, oc * 8 : oc * 8 + nh, :64
                ],
                ps[:, :ocs].rearrange("p (h c) -> p h c", c=64),
                bv_sb[:, oc * 512 : oc * 512 + ocs].rearrange(
                    "p (h c) -> p h c", c=64
                ),
                OP.add,
            )

        return emit

    _osb_state = {}

    def oproj_chunk(b, tt, oc):
        def emit():
            if oc == 0:
                _osb_state[(b, tt)] = osb_pool.tile(
                    [P, E], F32, tag="osb", name="osb"
                )
            osb = _osb_state[(b, tt)]
            ocs = 512 if oc == 0 else 256
            ps = psB.tile([P, CS], F32, tag="pb", name="pb")
            for ko in range(KO):
                nc.tensor.matmul(
                    ps[:, :ocs],
                    OT[b][:, ko, tt * P : (tt + 1) * P],
                    wproj_sb[:, ko, oc * 512 : oc * 512 + ocs],
                    start=(ko == 0),
                    stop=(ko == KO - 1),
                )
            nc.vector.tensor_tensor(
                osb[:, oc * 512 : oc * 512 + ocs],
                ps[:, :ocs],
                bp_sb[:, oc * 512 : oc * 512 + ocs],
                OP.add,
            )
            if oc == 1:
                nc.sync.dma_start(
                    out_d[b, tt * P : (tt + 1) * P, :], osb[:]
                )

        return emit

    # ---------------- attention (head pairs) ---------------------------
    def pair_scores_kt(b, pair, kt, psA, psB2, Pt0, Pt1):
        """4 score matmuls for head pair `pair` at key tile kt, alternating
        PE row-halves, then the two exps."""
        ft = pair
        for ch in range(NCH):
            for hh, ps in ((0, psA), (1, psB2)):
                pr = hh * 64
                nc.tensor.matmul(
                    ps[:, ch * CS : (ch + 1) * CS],
                    qkT[b][pr : pr + 64, KO + ft, kt * P : (kt + 1) * P],
                    qkT[b][pr : pr + 64, ft, ch * CS : (ch + 1) * CS],
                    start=True,
                    stop=True,
                )
        nc.scalar.activation(Pt0[:, kt, :], psA[:, :N], AF.Exp, scale=0.125)
        nc.scalar.activation(Pt1[:, kt, :], psB2[:, :N], AF.Exp, scale=0.125)

    def make_pv_pieces(b, pair, Pt0, Pt1):
        """PV matmuls + epilogues for both heads of `pair`, as closures
        consumed during the NEXT pair's score slots."""
        pieces = []
        state = {}

        def mk_mm(h, ch, g, Pt):
            def emit():
                if g == 0:
                    state[(h, ch)] = psV.tile(
                        [P, CS], F32, tag="po", name="po"
                    )
                po = state[(h, ch)]
                for kt in (2 * g, 2 * g + 1):
                    nc.tensor.matmul(
                        po[:65, :CS],
                        v_sb[b][:, kt, h * 65 : (h + 1) * 65],
                        Pt[:, kt, ch * CS : (ch + 1) * CS],
                        start=(kt == 0),
                        stop=(kt == TT - 1),
                    )

            return emit

        def mk_tail(h, ch, ft, pr):
            def emit():
                po = state[(h, ch)]
                otmp = small.tile([65, CS], BF16, tag="otmp", name="otmp")
                nc.vector.tensor_copy(otmp[:, :CS], po[0:65, :CS])
                nc.sync.dma_start(
                    sums_of[b][8 * h + 4 * ch : 8 * h + 4 * ch + 4, :],
                    otmp[64:65, :CS],
                )
                nc.sync.dma_start(
                    OT[b][pr : pr + 64, ft, ch * CS : (ch + 1) * CS],
                    otmp[0:64, :CS],
                )

            return emit

        for hh, Pt in ((0, Pt0), (1, Pt1)):
            h = 2 * pair + hh
            ft, pr = pair, hh * 64
            # ch interleaved inside each kt-group so Pt frees in kt order
            # (the next pair's exp WAR-waits on these reads)
            for g in range(TT // 2):
                for ch in range(NCH):
                    pieces.append(mk_mm(h, ch, g, Pt))
            for ch in range(NCH):
                pieces.append(mk_tail(h, ch, ft, pr))
        return pieces

    def normalize_head(b, h):
        """Closures: reciprocal of head h's denominators ([8,128] layout),
        then the OT multiply."""

        def recip():
            sums_sb = nrm.tile([8, P], BF16, tag="sums_sb", name="sums_sb")
            nc.sync.dma_start(sums_sb[:], sums_of[b][8 * h : 8 * h + 8, :])
            rsum = nrm.tile([8, P], BF16, tag="rsum", name="rsum")
            with nc.allow_low_precision(reason="softmax denom recip in bf16"):
                nc.vector.reciprocal(rsum[:], sums_sb[:])
            nc.sync.dma_start(rdram_of[b][8 * h : 8 * h + 8, :], rsum[:])

        def mult():
            ft, pr = h // 2, (h % 2) * 64
            rb = nrm2.tile([P, N], BF16, tag="rb", name="rb")
            nc.gpsimd.dma_start(
                rb[pr : pr + 64, :],
                rdram_of[b][8 * h : 8 * h + 8, :].partition_broadcast(64),
            )
            dst = OT[b][pr : pr + 64, ft, :]
            nc.vector.tensor_tensor(dst, dst, rb[pr : pr + 64, :], OP.mult)

        return [recip, mult]

    def alloc_attn_state(b):
        OT[b] = ot_pool.tile([P, KO, N], BF16, tag="OT", name="OT")
        sums_of[b] = dram.tile([8 * H, P], BF16, tag="sums", name="sums")
        rdram_of[b] = dram.tile([8 * H, P], BF16, tag="rdram", name="rdram")

    def pre_score_pair0(b, fill_iter):
        """Score pair 0 of batch b; pull fillers between kt slots."""
        psA = psS.tile([P, N], F32, tag="sA", name="sA")
        psB2 = psS.tile([P, N], F32, tag="sB", name="sB")
        Pt0 = pt_pool.tile([P, TT, N], BF16, tag="Pt", name="Pt")
        Pt1 = pt_pool.tile([P, TT, N], BF16, tag="Pt", name="Pt")
        for kt in range(TT):
            pair_scores_kt(b, 0, kt, psA, psB2, Pt0, Pt1)
            for _ in range(4):
                nxt = next(fill_iter, None)
                if nxt is not None:
                    nxt()
        return (psA, psB2, Pt0, Pt1)

    def emit_attention(b, hot, fillers, forced_map):
        """Pairs 1..NP-1 of batch b. `hot` = psum tiles + Pt of pair 0.
        fillers: list of closures, consumed to keep PE busy.
        forced_map: pair -> list of closures that MUST be consumed during
        that pair's slots (normalize work)."""
        psA, psB2, Pt0, Pt1 = hot
        fi = [0]

        def pop_filler():
            if fi[0] < len(fillers):
                fillers[fi[0]]()
                fi[0] += 1

        pieces = make_pv_pieces(b, 0, Pt0, Pt1)
        for pair in range(1, NP):
            Pt0 = pt_pool.tile([P, TT, N], BF16, tag="Pt", name="Pt")
            Pt1 = pt_pool.tile([P, TT, N], BF16, tag="Pt", name="Pt")
            npv = len(pieces)
            forced = list(forced_map.get(pair, ()))
            nfo = len(forced)
            pi = 0
            # spread remaining fillers evenly over remaining pairs
            nf_target = (len(fillers) - fi[0]) // (NP - pair)
            for kt in range(TT):
                pair_scores_kt(b, pair, kt, psA, psB2, Pt0, Pt1)
                quota = ((kt + 1) * npv) // TT - (kt * npv) // TT
                for _ in range(quota):
                    pieces[pi]()
                    pi += 1
                fq = ((kt + 1) * nfo) // TT - (kt * nfo) // TT
                for _ in range(fq):
                    forced.pop(0)()
                pq = ((kt + 1) * nf_target) // TT - (kt * nf_target) // TT
                for _ in range(pq):
                    pop_filler()
            while pi < npv:
                pieces[pi]()
                pi += 1
            pieces = make_pv_pieces(b, pair, Pt0, Pt1)
        # drain: PV of the last pair, normalize interleaved as soon as the
        # per-head tails land, remaining fillers last
        fNP = list(forced_map.get(NP, ()))
        fLast = list(forced_map.get(NP + 1, ()))
        for i, p in enumerate(pieces):
            p()
            if fNP:
                fNP.pop(0)()
            if i == 9 and len(fLast) >= 2:
                fLast.pop(0)()
                fLast.pop(0)()
            if i % 2 == 1:
                pop_filler()
        for c in fLast:
            c()
        while fi[0] < len(fillers):
            pop_filler()

    # ---------------- top-level schedule ------------------------------
    # batch 0 x prep first (x DMAs own the sync queue, PE transposes
    # start as soon as the first load+cast lands)
    bias_chunk()
    for tt in range(TT):
        xprep_chunk(0, tt)()
    # weights + qk projections for pairs 0 and 1: each m-tile unblocks
    # after one small [128,6,128] DMA on the (startup-idle) scalar queue
    for m in (0, KO, 1, KO + 1):
        wload(m, eng=nc.scalar)()
        for ch in range(NCH):
            qk_chunk(0, m, ch)()

    # order matters: all of v(0) must be EMITTED during pair0 pre-scoring
    # (32 slots), before PV(pair0) pieces read it in attention pair 1
    startup_fill = (
        [wload(2), qk_chunk(0, 2, 0), qk_chunk(0, 2, 1),
         wload(KO + 2), qk_chunk(0, KO + 2, 0), qk_chunk(0, KO + 2, 1)]
        + [wload(m) for m in range(12, 18)]
        + [v_chunk(0, tt, oc) for tt in range(TT) for oc in range(2)]
        + [wload(3), qk_chunk(0, 3, 0), qk_chunk(0, 3, 1),
           wload(KO + 3), qk_chunk(0, KO + 3, 0), qk_chunk(0, KO + 3, 1)]
        + [c for ft in (4, 5) for c in (
            wload(ft), qk_chunk(0, ft, 0), qk_chunk(0, ft, 1),
            wload(KO + ft), qk_chunk(0, KO + ft, 0), qk_chunk(0, KO + ft, 1))]
        + [wload(m) for m in range(18, 24)]
    )
    sf_iter = iter(startup_fill)
    alloc_attn_state(0)
    hot0 = pre_score_pair0(0, sf_iter)

    for b in range(n_batch):
        fillers = list(sf_iter) if b == 0 else []
        sf_iter = iter(())
        if b + 1 < n_batch:
            fillers += [xprep_chunk(b + 1, tt) for tt in range(TT)]
            fillers += [
                qk_chunk(b + 1, m, ch)
                for ft in range(KO)
                for m in (ft, KO + ft)
                for ch in range(NCH)
            ]
        if b > 0:
            fillers += [
                oproj_chunk(b - 1, tt, oc) for tt in range(TT) for oc in range(2)
            ]
        # normalize pair p's heads during pair p+2's slots
        forced_map = {}
        for pair in range(NP):
            forced_map.setdefault(min(pair + 2, NP + 1), []).extend(
                normalize_head(b, 2 * pair) + normalize_head(b, 2 * pair + 1)
            )
        emit_attention(b, hot0, fillers, forced_map)
        if b + 1 < n_batch:
            # bridge: pre-score pair0 of b+1 while vproj(b+1) streams
            alloc_attn_state(b + 1)
            bridge = [v_chunk(b + 1, tt, oc) for tt in range(TT) for oc in range(2)]
            hot0 = pre_score_pair0(b + 1, iter(bridge))
    # tail: out projection of the last batch, split so the ko=0..4 streams
    # run inside the last heads' normalize latency window and only the
    # ko=5 accumulate (needs heads 10/11 normalized) waits on it
    bl = n_batch - 1
    tail_ps = {}

    def op_part1(tt, oc):
        ocs = 512 if oc == 0 else 256
        ps = psB.tile([P, CS], F32, tag="pb", name="pb")
        tail_ps[(tt, oc)] = ps
        for ko in range(KO - 1):
            nc.tensor.matmul(
                ps[:, :ocs],
                OT[bl][:, ko, tt * P : (tt + 1) * P],
                wproj_sb[:, ko, oc * 512 : oc * 512 + ocs],
                start=(ko == 0),
                stop=False,
            )

    def op_part2(tt, oc):
        if oc == 0:
            _osb_state[(bl, tt)] = osb_pool.tile([P, E], F32, tag="osb", name="osb")
        osb = _osb_state[(bl, tt)]
        ocs = 512 if oc == 0 else 256
        ps = tail_ps[(tt, oc)]
        nc.tensor.matmul(
            ps[:, :ocs],
            OT[bl][:, KO - 1, tt * P : (tt + 1) * P],
            wproj_sb[:, KO - 1, oc * 512 : oc * 512 + ocs],
            start=False,
            stop=True,
        )
        nc.vector.tensor_tensor(
            osb[:, oc * 512 : oc * 512 + ocs],
            ps[:, :ocs],
            bp_sb[:, oc * 512 : oc * 512 + ocs],
            OP.add,
        )
        if oc == 1:
            nc.sync.dma_start(out_d[bl, tt * P : (tt + 1) * P, :], osb[:])

    op_part1(0, 0)
    op_part1(0, 1)
    for tt in range(TT):
        op_part2(tt, 0)
        op_part2(tt, 1)
        if tt + 1 < TT:
            op_part1(tt + 1, 0)
            op_part1(tt + 1, 1)
    if dbg is not None:
        nc.sync.dma_start(dbg["qkT"], qkT[0][:])
        nc.sync.dma_start(dbg["v"], v_sb[0][:])
        nc.sync.dma_start(dbg["ot"], OT[0][:])
        nc.sync.dma_start(dbg["sums"], sums_of[0][:])


def build_graph(n_batch=BPC, N=N_FULL, n_cores=N_CORES, debug=False):
    nc = bacc.Bacc(
        "TRN2", target_bir_lowering=False, debug=False, num_devices=n_cores
    )
    x_d = nc.dram_tensor("x", [n_batch, N, E], F32, kind="ExternalInput").ap()
    wqkv_d = nc.dram_tensor("w_qkv", [E, 3 * E], F32, kind="ExternalInput").ap()
    bqkv_d = nc.dram_tensor("b_qkv", [3 * E], F32, kind="ExternalInput").ap()
    wproj_d = nc.dram_tensor("w_proj", [E, E], F32, kind="ExternalInput").ap()
    bproj_d = nc.dram_tensor("b_proj", [E], F32, kind="ExternalInput").ap()
    out_d = nc.dram_tensor("out", [n_batch, N, E], F32, kind="ExternalOutput").ap()

    dbg = None
    if debug:
        dbg = {
            "qkT": nc.dram_tensor("dbg_qkT", [P, 2 * KO, N], BF16, kind="ExternalOutput").ap(),
            "v": nc.dram_tensor("dbg_v", [P, N_FULL // P, H * 65], BF16, kind="ExternalOutput").ap(),
            "ot": nc.dram_tensor("dbg_ot", [P, KO, N], BF16, kind="ExternalOutput").ap(),
            "sums": nc.dram_tensor("dbg_sums", [8 * H, P], BF16, kind="ExternalOutput").ap(),
        }
    with tile.TileContext(nc) as tc, ExitStack() as ctx:
        _body(ctx, tc, out_d, x_d, wqkv_d, bqkv_d, wproj_d, bproj_d, n_batch, N, dbg=dbg)
    nc.compile()
    return nc


_NC_CACHE = {}


def _get_graph():
    if "nc" not in _NC_CACHE:
        _NC_CACHE["nc"] = build_graph()
    return _NC_CACHE["nc"]


def run_on_hw(x, w_qkv, b_qkv, w_proj, b_proj, trace=False):
    nc = _get_graph()
    x = np.ascontiguousarray(np.asarray(x, dtype=np.float32))
    shared = {
        "w_qkv": np.ascontiguousarray(np.asarray(w_qkv, dtype=np.float32)),
        "b_qkv": np.ascontiguousarray(np.asarray(b_qkv, dtype=np.float32)),
        "w_proj": np.ascontiguousarray(np.asarray(w_proj, dtype=np.float32)),
        "b_proj": np.ascontiguousarray(np.asarray(b_proj, dtype=np.float32)),
    }
    in_maps = [
        {"x": x[i * BPC : (i + 1) * BPC], **shared} for i in range(N_CORES)
    ]
    res = run_bass_kernel_spmd(
        nc, in_maps, core_ids=list(range(N_CORES)), trace=trace
    )
    out = np.concatenate([r["out"] for r in res.results], axis=0)
    return out, res


def kernel(x, w_qkv, b_qkv, w_proj, b_proj):
    out, _ = run_on_hw(x, w_qkv, b_qkv, w_proj, b_proj)
    return out
